# revision 15
# baseline (speedup 1.0000x reference)
"""Trainium2 Bass kernel for nn_Attention_10711648436709.

Math (faithful to reference):
    h = einsum('bhik,bhjk->bhij', Q, K) / sqrt(H)     # scale = sqrt(16) = 4
    w = softmax(h, axis=0)                            # over the BATCH axis (B=4)
    out = einsum('bhij,bhjv->bhiv', w, V)
    (mask is a no-op in the reference)

Sharding: head-parallel across 8 cores (16 heads -> 2 heads/core).
Softmax over batch stays core-local => communication-free.

Per-core layout: compute transposed scores S^T[j, i]:
 - QK:  lhsT = [K_b^T ; K_0^T][d, j-block]  rhs = [Q_b^T ; -Q_0^T][d, i-chunk]
   (host packs/negates) -> one K=128 matmul per b in {1,2,3} yields
   g_b = h_b - h_0 directly (batch-0-pivot softmax).
 - w_b = e^{g_b/4} * r, w_0 = r, r = 1/(1 + sum_b e^{g_b/4}).
 - PV:  lhsT = V[j-block, v]  rhs = W_b[j, i-chunk]; out^T[v,i] accumulates
   in PSUM over the 16 j-blocks; host transposes back.

Engine split (v2): exp on ACT; E1+E2 and the W=E*r multiply on DVE (2x bf16);
den = (E3+1)+T1 on GpSimd; reciprocal via the fused RECIPROCAL_APPROX_FAST
custom DVE op (1 instr) with a fraction routed to ACT (Ln+Exp) for balance.
Elementwise ops batched over PAIRS of j-blocks ([128,1024] tiles) to amortize
per-instruction overheads. Emission is software-pipelined: QK runs 2 pairs
ahead of PV so the tensor engine stays continuously fed (p-state ramp).
"""

import sys
import os

for p in ("/opt/trn_rl_repo",):
    if p not in sys.path:
        sys.path.insert(0, p)

import numpy as np
import ml_dtypes

B, H, S, D = 4, 16, 2048, 64
NCORES = 8
HL = H // NCORES          # 2 heads per core
NB = S // 128             # 16 j-blocks
NI = S // 512             # 4 i-chunks
NPAIR = NB // 2           # 8 j-block pairs per (hl, ic) round
NROUND = HL * NI          # 8 rounds
NGP = NROUND * NPAIR      # 64 global pairs

# every ACT_EVERY-th pair computes r on ACT (Ln+Exp) instead of DVE
ACT_EVERY = 4

TRACE = False
LAST_EXEC_NS = None
LAST_RESULTS = None

_NC = None


def _build_nc():
    import concourse.bass as bass
    import concourse.mybir as mybir
    import concourse.tile as tile
    from concourse.dve_ops import (
        RECIP_APPROX_FAST_CONSTS,
        RECIPROCAL_APPROX_FAST,
    )

    DT = mybir.dt
    AF = mybir.ActivationFunctionType
    ALU = mybir.AluOpType
    RC = RECIP_APPROX_FAST_CONSTS

    nc = bass.Bass()
    qt = nc.declare_dram_parameter("qt", [3, HL, 128, S], DT.bfloat16, isOutput=False)
    kt = nc.declare_dram_parameter("kt", [3, HL, 128, S], DT.bfloat16, isOutput=False)
    vv = nc.declare_dram_parameter("v", [B, HL, S, D], DT.bfloat16, isOutput=False)
    out = nc.declare_dram_parameter("out", [B, HL, D, S], DT.float32, isOutput=True)

    with tile.TileContext(nc) as tc:
        with (
            tc.tile_pool(name="inputs", bufs=1) as ipool,
            tc.tile_pool(name="work", bufs=5) as wpool,
            # rb lives until PV consumes it PV_LAG pairs later -> needs
            # bufs > PV_LAG + chain depth or the recip chain WAR-stalls on PV
            tc.tile_pool(name="mid", bufs=5) as mpool,
            tc.tile_pool(name="outsb", bufs=4) as opool,
            tc.tile_pool(name="qkps", bufs=2, space="PSUM") as qkpool,
            tc.tile_pool(name="ops", bufs=1, space="PSUM") as opsum,
        ):
            QT = ipool.tile([128, 3 * HL * S], DT.bfloat16, tag="qt")
            KT = ipool.tile([128, 3 * HL * S], DT.bfloat16, tag="kt")
            VA = ipool.tile([128, B * HL * NB * D], DT.bfloat16, tag="va")
            # hl-major so the first (hl=0) round's operands land first
            for hl in range(HL):
                for bb in range(3):
                    off = (bb * HL + hl) * S
                    nc.sync.dma_start(out=KT[:, off : off + S], in_=kt[bb, hl])
                    nc.sync.dma_start(out=QT[:, off : off + S], in_=qt[bb, hl])
                for b in range(B):
                    voff = (b * HL + hl) * NB * D
                    nc.sync.dma_start(
                        out=VA[:, voff : voff + NB * D].rearrange(
                            "p (n d) -> p n d", d=D
                        ),
                        in_=vv[b, hl].rearrange("(n p) d -> p n d", p=128),
                    )

            # per-global-pair state for the software pipeline
            st = {}

            def rnd(gp):
                r = gp // NPAIR
                return r // NI, r % NI  # (hl, ic)

            def stage_qk(gp):
                """QK matmuls + exp for both j-blocks of pair gp."""
                hl, ic = rnd(gp)
                p = gp % NPAIR
                # E pair tile, plane-major: [b=3, pair-interleaved 1024]
                E = wpool.tile([128, 3072], DT.bfloat16, tag="E")
                for half in range(2):
                    jb = 2 * p + half
                    qk = qkpool.tile([128, 1536], DT.float32, tag="qk")
                    for bb in range(3):
                        off = (bb * HL + hl) * S
                        nc.tensor.matmul(
                            qk[:, bb * 512 : (bb + 1) * 512],
                            lhsT=KT[:, off + jb * 128 : off + jb * 128 + 128],
                            rhs=QT[:, off + ic * 512 : off + ic * 512 + 512],
                            start=True,
                            stop=True,
                        )
                    # exp of all 3 planes into interleaved pair layout:
                    # plane b lives at columns [b*1024 + half*512, +512)
                    E3 = E.rearrange("q (b n) -> q b n", b=3)
                    nc.scalar.activation(
                        E3[:, :, half * 512 : half * 512 + 512],
                        qk.rearrange("q (b n) -> q b n", b=3),
                        AF.Exp,
                        scale=0.25,
                    )
                st[gp] = {"E": E}

            def stage_ew(gp):
                """T1/den/recip/W for pair gp (runs one pair behind QK)."""
                E = st[gp]["E"]
                T1 = mpool.tile([128, 1024], DT.bfloat16, tag="T1")
                nc.vector.tensor_add(T1, E[:, 0:1024], E[:, 1024:2048])
                # fold the softmax +1 in before the (slow) gpsimd hop so the
                # gpsimd add feeds the reciprocal directly
                T2 = mpool.tile([128, 1024], DT.bfloat16, tag="T2")
                nc.vector.tensor_scalar(
                    out=T2,
                    in0=T1,
                    scalar1=1.0,
                    scalar2=None,
                    op0=ALU.add,
                )
                # GpSimd supports plain TensorTensor only (no STT, no PSUM)
                den = mpool.tile([128, 1024], DT.bfloat16, tag="den")
                nc.gpsimd.tensor_add(den, E[:, 2048:3072], T2)
                rb = mpool.tile([128, 1024], DT.bfloat16, tag="rb")
                if gp % ACT_EVERY == ACT_EVERY - 1:
                    # reciprocal on ACT: r = exp(-ln(den)); Ln and Exp share
                    # the natural_log_exp table set
                    lnt = mpool.tile([128, 1024], DT.bfloat16, tag="lnt")
                    nc.scalar.activation(lnt, den, AF.Ln)
                    nc.scalar.activation(rb, lnt, AF.Exp, scale=-1.0)
                else:
                    # fused seed + 2 Newton passes, one DVE instruction
                    nc.vector._custom_dve(
                        RECIPROCAL_APPROX_FAST,
                        out=rb,
                        in0=den,
                        s0=RC["s0"],
                        s1=RC["s1"],
                        imm2=RC["imm2"],
                    )
                W = wpool.tile([128, 3072], DT.bfloat16, tag="W")
                nc.vector.tensor_mul(
                    W.rearrange("q (b n) -> q b n", b=3),
                    E.rearrange("q (b n) -> q b n", b=3),
                    rb.unsqueeze(1).broadcast_to([128, 3, 1024]),
                )
                st[gp].update({"rb": rb, "W": W})

            def stage_pv(gp, po):
                """PV accumulation for both j-blocks of pair gp."""
                hl, ic = rnd(gp)
                p = gp % NPAIR
                rb, W = st[gp]["rb"], st[gp]["W"]
                for half in range(2):
                    jb = 2 * p + half
                    rhss = [
                        rb[:, half * 512 : half * 512 + 512],
                        W[:, half * 512 : half * 512 + 512],
                        W[:, 1024 + half * 512 : 1024 + half * 512 + 512],
                        W[:, 2048 + half * 512 : 2048 + half * 512 + 512],
                    ]
                    for pp in range(2):
                        for bhalf in range(2):
                            b = 2 * pp + bhalf
                            voff = (b * HL + hl) * NB * D + jb * D
                            nc.tensor.matmul(
                                po[pp][64 * bhalf : 64 * (bhalf + 1), :],
                                lhsT=VA[:, voff : voff + D],
                                rhs=rhss[b],
                                start=(jb == 0),
                                stop=(jb == NB - 1),
                                tile_position=(0, 64 * bhalf),
                            )
                del st[gp]

            def flush_round(r, po):
                """PSUM -> SBUF -> HBM for round r's outputs."""
                hl, ic = r // NI, r % NI
                for pp in range(2):
                    osb = opool.tile([128, 512], DT.float32, tag=f"osb{pp}")
                    # ACT, not GpSimd: GPSIMD instructions cannot access PSUM
                    nc.scalar.copy(osb, po[pp])
                    for bhalf in range(2):
                        b = 2 * pp + bhalf
                        nc.sync.dma_start(
                            out=out[b, hl, :, ic * 512 : (ic + 1) * 512],
                            in_=osb[64 * bhalf : 64 * (bhalf + 1), :],
                        )

            # software-pipelined emission: QK(gp) | EW(gp-1) | PV(gp-PV_LAG)
            PV_LAG = 3
            po_by_round = {}
            for gp in range(NGP + PV_LAG):
                if gp < NGP:
                    stage_qk(gp)
                if 0 <= gp - 1 < NGP:
                    stage_ew(gp - 1)
                pv = gp - PV_LAG
                if pv >= 0:
                    r = pv // NPAIR
                    if pv % NPAIR == 0:
                        po_by_round[r] = [
                            opsum.tile(
                                [128, 512], DT.float32, tag=f"po{q}", name=f"po{q}"
                            )
                            for q in range(2)
                        ]
                    stage_pv(pv, po_by_round[r])
                    if pv % NPAIR == NPAIR - 1:
                        flush_round(r, po_by_round.pop(r))

    # populate .instr bytes for InstISA subclasses (the custom DVE op);
    # without this the NEFF compiler sees empty .instr -> "ISA wrong length"
    from concourse.library_overlay import lower_extended_insts

    lower_extended_insts(nc)
    return nc


def _patch_bir_waits(bir_json: bytes) -> bytes:
    """This walrus build only accepts 1 sync wait per instruction (2 for
    DMACopy); Tile emits more. Legalize:
      1. merge duplicate-semaphore waits (keep max threshold),
      2. drop waits that are transitively implied (vector-clock replay over
         the straight-line program: in-order completion per engine, FIFO per
         DMA queue, and the knowledge a producer had when it bumped a sem),
      3. split any residual multi-wait onto injected EventSemaphore
         instructions on the same engine right before the instruction.
    Only monotonic sem-inc/sem-ge-imm semaphores participate in (2); barrier
    sems (dec/eq) are left untouched."""
    import json
    from collections import defaultdict

    bir = json.loads(bir_json)

    for fn in bir["functions"]:
        insts = []
        for bb in fn["blocks"]:
            for inst in bb.get("instructions", []):
                insts.append(inst)

        # classify sems: monotonic = all updates are positive sem-inc and
        # all waits are sem-ge-imm
        bad_sems = set()
        for inst in insts:
            si = inst.get("sync_info") or {}
            for u in si.get("on_update") or []:
                if u.get("update_mode") != "sem-inc" or u.get("update_value", 0) <= 0:
                    bad_sems.add(u["id"])
            for w in si.get("on_wait") or []:
                if w.get("wait_mode") != "sem-ge-imm":
                    bad_sems.add(w["id"])

        # proc of an instruction: its engine stream, except DMACopy whose
        # completion (and sem update) is FIFO per DMA queue, keyed by the
        # sem it updates.
        def proc_of(inst):
            if inst.get("opcode") == "DMACopy":
                si = inst.get("sync_info") or {}
                ups = si.get("on_update") or []
                if ups:
                    return ("dma", ups[0]["id"])
            return ("eng", inst.get("engine"))

        sem_val = defaultdict(int)          # current cumulative value per sem
        producers = defaultdict(list)       # sem -> [(value_after, CK dict)]
        know = defaultdict(dict)            # proc -> {sem: guaranteed min}

        def join(dst, src):
            for s, v in src.items():
                if dst.get(s, 0) < v:
                    dst[s] = v

        out_blocks = {id(bb): [] for bb in fn["blocks"]}
        inj = 0
        for bb in fn["blocks"]:
            new_list = []
            for inst in bb.get("instructions", []):
                p = proc_of(inst)
                eng_p = ("eng", inst.get("engine"))
                # waits on a DMACopy are enforced by the DGE queue (FIFO per
                # queue), not the issuing engine — track knowledge per queue
                kp = p if p[0] == "dma" else eng_p
                si = inst.get("sync_info") or {}
                waits = si.get("on_wait") or []
                # merge duplicate sems
                merged = {}
                for w in waits:
                    k = w["id"]
                    if k not in merged or w.get("wait_value", 0) > merged[k].get(
                        "wait_value", 0
                    ):
                        merged[k] = w
                waits = list(merged.values())
                kept = []
                for w in waits:
                    s, v = w["id"], w.get("wait_value", 0)
                    if s in bad_sems:
                        kept.append(w)
                        continue
                    if know[kp].get(s, 0) >= v:
                        continue  # redundant
                    kept.append(w)
                    know[kp][s] = max(know[kp].get(s, 0), v)
                    # transitive knowledge from the producer that reached v
                    for val_after, ck in producers[s]:
                        if val_after >= v:
                            join(know[kp], ck)
                            break
                # split if too many waits remain
                budget = 1
                while len(kept) > budget:
                    w = kept.pop(0)
                    inj += 1
                    new_list.append(
                        {
                            "debug": inst.get("debug", 0),
                            "engine": inst.get("engine"),
                            "ins": [],
                            "name": f"WS-{inj}-{inst.get('name')}",
                            "opcode": "EventSemaphore",
                            "outs": [],
                            "sync_info": {"on_update": [], "on_wait": [w]},
                        }
                    )
                si["on_wait"] = kept
                inst["sync_info"] = si
                new_list.append(inst)
                # apply this instruction's updates for downstream knowledge
                ups = si.get("on_update") or []
                ck = None
                for u in ups:
                    s = u["id"]
                    if s in bad_sems:
                        continue
                    sem_val[s] += u.get("update_value", 0)
                    if ck is None:
                        # completion knowledge: what this proc knew here
                        # (for DMA: queue knowledge + engine state at issue)
                        ck = dict(know[kp])
                        if p[0] == "dma":
                            join(ck, know[eng_p])
                    ck[s] = sem_val[s]
                    producers[s].append((sem_val[s], ck))
                # a proc knows its own sems' values after completion
                if p[0] == "eng":
                    for u in ups:
                        if u["id"] not in bad_sems:
                            know[eng_p][u["id"]] = sem_val[u["id"]]
            out_blocks[id(bb)] = new_list
        for bb in fn["blocks"]:
            bb["instructions"] = out_blocks[id(bb)]
    return json.dumps(bir).encode()


_PATCHED = False


def _install_bir_patch():
    global _PATCHED
    if _PATCHED:
        return
    import concourse.bass2jax as bass2jax
    from concourse import bass_utils as _bu

    orig = _bu.compile_bir_kernel

    def patched(bir_json, tmpdir, neff_name="file.neff"):
        return orig(_patch_bir_waits(bir_json), tmpdir, neff_name)

    bass2jax.compile_bir_kernel = patched
    # keep profile artifacts local — no bucket in this environment
    _bu.upload_artifacts = lambda tmpdir: str(tmpdir)
    _PATCHED = True


def _install_ntff_shim():
    """run_bass_kernel_spmd(trace=True) under axon needs
    antenv.axon_hooks.get_axon_ntff_profile_hook; the module isn't staged in
    this image, but libaxon_pjrt.so exposes the profile C ABI — recreate the
    shim (same recipe as trn_agent_boot)."""
    import sys as _sys

    if "antenv.axon_hooks" in _sys.modules:
        return
    import contextlib
    import ctypes
    import types

    import antenv  # noqa: F401

    so_path = "/opt/axon/libaxon_pjrt.so"
    hook = None
    try:
        lib = ctypes.CDLL(so_path)
        if hasattr(lib, "axon_start_nrt_profile"):
            lib.axon_start_nrt_profile.argtypes = [
                ctypes.POINTER(ctypes.c_int64),
                ctypes.c_size_t,
            ]
            lib.axon_start_nrt_profile.restype = ctypes.c_int64
            lib.axon_stop_nrt_profile.argtypes = [ctypes.c_char_p]
            lib.axon_stop_nrt_profile.restype = ctypes.c_int64

            @contextlib.contextmanager
            def hook(output_dir, device_ids):
                import jax

                jax.devices()
                if device_ids:
                    ids = (ctypes.c_int64 * len(device_ids))(*device_ids)
                    rc = lib.axon_start_nrt_profile(ids, len(device_ids))
                else:
                    rc = lib.axon_start_nrt_profile(None, 0)
                if rc != 0:
                    raise RuntimeError(f"axon_start_nrt_profile rc={rc}")
                try:
                    yield
                finally:
                    n = lib.axon_stop_nrt_profile(str(output_dir).encode())
                    print(
                        f"ntff profile: {n} file(s) -> {output_dir}",
                        file=_sys.stderr,
                    )
    except OSError:
        pass

    mod = types.ModuleType("antenv.axon_hooks")
    mod.get_axon_ntff_profile_hook = lambda: hook
    mod.set_axon_ntff_profile_hook = lambda h: None
    _sys.modules["antenv.axon_hooks"] = mod
    import antenv as _ae

    _ae.axon_hooks = mod


def kernel(query, key, value, mask=None):
    global _NC, LAST_EXEC_NS, LAST_RESULTS
    from concourse.bass_utils import run_bass_kernel_spmd

    _install_bir_patch()
    if TRACE:
        _install_ntff_shim()

    query = np.asarray(query, dtype=np.float32)
    key = np.asarray(key, dtype=np.float32)
    value = np.asarray(value, dtype=np.float32)

    if _NC is None:
        _NC = _build_nc()
    nc = _NC

    bf16 = ml_dtypes.bfloat16

    def pack_pivot(x, negate_base):
        # [B, HL, S, D] -> [B, HL, D, S]; stack [x_b^T ; (+-)x_0^T] on the
        # partition axis for b = 1..3 -> [3, HL, 128, S]
        xt = x.transpose(0, 1, 3, 2)  # [B, HL, D, S]
        base = -xt[0] if negate_base else xt[0]  # [HL, D, S]
        stk = np.stack(
            [np.concatenate([xt[b], base], axis=1) for b in (1, 2, 3)], axis=0
        )
        return np.ascontiguousarray(stk).astype(bf16)

    in_maps = []
    for c in range(NCORES):
        hs = slice(HL * c, HL * (c + 1))
        qtc = pack_pivot(query[:, hs], negate_base=True)
        ktc = pack_pivot(key[:, hs], negate_base=False)
        vc = np.ascontiguousarray(value[:, hs]).astype(bf16)
        in_maps.append({"qt": qtc, "kt": ktc, "v": vc})

    res = run_bass_kernel_spmd(
        nc, in_maps, core_ids=list(range(NCORES)), trace=TRACE
    )
    LAST_RESULTS = res
    LAST_EXEC_NS = getattr(res, "exec_time_ns", None)

    full = np.empty((B, H, S, D), dtype=np.float32)
    for c in range(NCORES):
        o = np.asarray(res.results[c]["out"])  # [B, HL, D, S]
        full[:, HL * c : HL * (c + 1)] = o.transpose(0, 1, 3, 2)
    return full


# revision 19
# speedup vs baseline: 1.1866x; 1.1866x over previous
"""Trainium2 Bass kernel for nn_Attention_10711648436709.

Math (faithful to reference):
    h = einsum('bhik,bhjk->bhij', Q, K) / sqrt(H)     # scale = sqrt(16) = 4
    w = softmax(h, axis=0)                            # over the BATCH axis (B=4)
    out = einsum('bhij,bhjv->bhiv', w, V)
    (mask is a no-op in the reference)

Sharding: head-parallel across 8 cores (16 heads -> 2 heads/core).
Softmax over batch stays core-local => communication-free.

Per-core layout: compute transposed scores S^T[j, i]:
 - QK:  lhsT = [K_b^T ; K_0^T][d, j-block]  rhs = [Q_b^T ; -Q_0^T][d, i-chunk]
   (host packs/negates) -> one K=128 matmul per b in {1,2,3} yields
   g_b = h_b - h_0 directly (batch-0-pivot softmax).
 - w_b = e^{g_b/4} * r, w_0 = r, r = 1/(1 + sum_b e^{g_b/4}).
 - PV:  lhsT = V[j-block, v]  rhs = W_b[j, i-chunk]; out^T[v,i] accumulates
   in PSUM over the 16 j-blocks; host transposes back.

Engine split (v2): exp on ACT; E1+E2 and the W=E*r multiply on DVE (2x bf16);
den = (E3+1)+T1 on GpSimd; reciprocal via the fused RECIPROCAL_APPROX_FAST
custom DVE op (1 instr) with a fraction routed to ACT (Ln+Exp) for balance.
Elementwise ops batched over PAIRS of j-blocks ([128,1024] tiles) to amortize
per-instruction overheads. Emission is software-pipelined: QK runs 2 pairs
ahead of PV so the tensor engine stays continuously fed (p-state ramp).
"""

import sys
import os

for p in ("/opt/trn_rl_repo",):
    if p not in sys.path:
        sys.path.insert(0, p)

import numpy as np
import ml_dtypes

B, H, S, D = 4, 16, 2048, 64
NCORES = 8
HL = H // NCORES          # 2 heads per core
NB = S // 128             # 16 j-blocks
NI = S // 512             # 4 i-chunks
NPAIR = NB // 2           # 8 j-block pairs per (hl, ic) round
NROUND = HL * NI          # 8 rounds
NGP = NROUND * NPAIR      # 64 global pairs

# every ACT_EVERY-th pair computes r on ACT (Ln+Exp) instead of DVE
ACT_EVERY = 6

TRACE = False
LAST_EXEC_NS = None
LAST_RESULTS = None

_NC = None


def _build_nc():
    import concourse.bass as bass
    import concourse.mybir as mybir
    import concourse.tile as tile
    from concourse.dve_ops import (
        RECIP_APPROX_FAST_CONSTS,
        RECIPROCAL_APPROX_FAST,
    )

    DT = mybir.dt
    AF = mybir.ActivationFunctionType
    ALU = mybir.AluOpType
    RC = RECIP_APPROX_FAST_CONSTS

    nc = bass.Bass()
    qt = nc.declare_dram_parameter("qt", [3, HL, 128, S], DT.bfloat16, isOutput=False)
    kt = nc.declare_dram_parameter("kt", [3, HL, 128, S], DT.bfloat16, isOutput=False)
    vv = nc.declare_dram_parameter("v", [B, HL, S, D], DT.bfloat16, isOutput=False)
    out = nc.declare_dram_parameter("out", [B, HL, D, S], DT.float32, isOutput=True)

    with tile.TileContext(nc) as tc:
        with (
            tc.tile_pool(name="inputs", bufs=1) as ipool,
            tc.tile_pool(name="work", bufs=6) as wpool,
            # rb lives until PV consumes it PV_LAG pairs later -> needs
            # bufs > PV_LAG + chain depth or the recip write WAR-stalls on PV
            tc.tile_pool(name="rbp", bufs=6) as rbpool,
            tc.tile_pool(name="mid", bufs=3) as mpool,
            tc.tile_pool(name="lnp", bufs=2) as lnpool,
            tc.tile_pool(name="outsb", bufs=4) as opool,
            tc.tile_pool(name="qkps", bufs=2, space="PSUM") as qkpool,
            tc.tile_pool(name="ops", bufs=1, space="PSUM") as opsum,
        ):
            QT = ipool.tile([128, 3 * HL * S], DT.bfloat16, tag="qt")
            KT = ipool.tile([128, 3 * HL * S], DT.bfloat16, tag="kt")
            VA = ipool.tile([128, B * HL * NB * D], DT.bfloat16, tag="va")
            # hl-major so the first (hl=0) round's operands land first
            for hl in range(HL):
                for bb in range(3):
                    off = (bb * HL + hl) * S
                    nc.sync.dma_start(out=KT[:, off : off + S], in_=kt[bb, hl])
                    nc.sync.dma_start(out=QT[:, off : off + S], in_=qt[bb, hl])
                for b in range(B):
                    voff = (b * HL + hl) * NB * D
                    nc.sync.dma_start(
                        out=VA[:, voff : voff + NB * D].rearrange(
                            "p (n d) -> p n d", d=D
                        ),
                        in_=vv[b, hl].rearrange("(n p) d -> p n d", p=128),
                    )

            # per-global-pair state for the software pipeline
            st = {}

            def rnd(gp):
                r = gp // NPAIR
                return r // NI, r % NI  # (hl, ic)

            def stage_qk(gp):
                """QK matmuls + exp for both j-blocks of pair gp."""
                hl, ic = rnd(gp)
                p = gp % NPAIR
                # E pair tile, plane-major: [b=3, pair-interleaved 1024]
                E = wpool.tile([128, 3072], DT.bfloat16, tag="E")
                for half in range(2):
                    jb = 2 * p + half
                    qk = qkpool.tile([128, 1536], DT.float32, tag="qk")
                    for bb in range(3):
                        off = (bb * HL + hl) * S
                        nc.tensor.matmul(
                            qk[:, bb * 512 : (bb + 1) * 512],
                            lhsT=KT[:, off + jb * 128 : off + jb * 128 + 128],
                            rhs=QT[:, off + ic * 512 : off + ic * 512 + 512],
                            start=True,
                            stop=True,
                        )
                    # exp of all 3 planes into interleaved pair layout:
                    # plane b lives at columns [b*1024 + half*512, +512)
                    E3 = E.rearrange("q (b n) -> q b n", b=3)
                    nc.scalar.activation(
                        E3[:, :, half * 512 : half * 512 + 512],
                        qk.rearrange("q (b n) -> q b n", b=3),
                        AF.Exp,
                        scale=0.25,
                    )
                st[gp] = {"E": E}

            def stage_ew(gp):
                """T1/den/recip/W for pair gp (runs one pair behind QK)."""
                E = st[gp]["E"]
                T1 = mpool.tile([128, 1024], DT.bfloat16, tag="T1")
                nc.vector.tensor_add(T1, E[:, 0:1024], E[:, 1024:2048])
                # fold the softmax +1 in before the (slow) gpsimd hop so the
                # gpsimd add feeds the reciprocal directly
                T2 = mpool.tile([128, 1024], DT.bfloat16, tag="T2")
                nc.vector.tensor_scalar(
                    out=T2,
                    in0=T1,
                    scalar1=1.0,
                    scalar2=None,
                    op0=ALU.add,
                )
                # GpSimd supports plain TensorTensor only (no STT, no PSUM)
                den = mpool.tile([128, 1024], DT.bfloat16, tag="den")
                nc.gpsimd.tensor_add(den, E[:, 2048:3072], T2)
                rb = rbpool.tile([128, 1024], DT.bfloat16, tag="rb")
                if gp % ACT_EVERY == ACT_EVERY - 1:
                    # reciprocal on ACT: r = exp(-ln(den)); Ln and Exp share
                    # the natural_log_exp table set
                    lnt = lnpool.tile([128, 1024], DT.bfloat16, tag="lnt")
                    nc.scalar.activation(lnt, den, AF.Ln)
                    nc.scalar.activation(rb, lnt, AF.Exp, scale=-1.0)
                else:
                    # fused seed + 2 Newton passes, one DVE instruction
                    nc.vector._custom_dve(
                        RECIPROCAL_APPROX_FAST,
                        out=rb,
                        in0=den,
                        s0=RC["s0"],
                        s1=RC["s1"],
                        imm2=RC["imm2"],
                    )
                W = wpool.tile([128, 3072], DT.bfloat16, tag="W")
                nc.vector.tensor_mul(
                    W.rearrange("q (b n) -> q b n", b=3),
                    E.rearrange("q (b n) -> q b n", b=3),
                    rb.unsqueeze(1).broadcast_to([128, 3, 1024]),
                )
                st[gp].update({"rb": rb, "W": W})

            def stage_pv(gp, po):
                """PV accumulation for both j-blocks of pair gp."""
                hl, ic = rnd(gp)
                p = gp % NPAIR
                rb, W = st[gp]["rb"], st[gp]["W"]
                for half in range(2):
                    jb = 2 * p + half
                    rhss = [
                        rb[:, half * 512 : half * 512 + 512],
                        W[:, half * 512 : half * 512 + 512],
                        W[:, 1024 + half * 512 : 1024 + half * 512 + 512],
                        W[:, 2048 + half * 512 : 2048 + half * 512 + 512],
                    ]
                    for pp in range(2):
                        for bhalf in range(2):
                            b = 2 * pp + bhalf
                            voff = (b * HL + hl) * NB * D + jb * D
                            nc.tensor.matmul(
                                po[pp][64 * bhalf : 64 * (bhalf + 1), :],
                                lhsT=VA[:, voff : voff + D],
                                rhs=rhss[b],
                                start=(jb == 0),
                                stop=(jb == NB - 1),
                                tile_position=(0, 64 * bhalf),
                            )
                del st[gp]

            def flush_round(r, po):
                """PSUM -> SBUF -> HBM for round r's outputs."""
                hl, ic = r // NI, r % NI
                for pp in range(2):
                    osb = opool.tile([128, 512], DT.float32, tag=f"osb{pp}")
                    # ACT, not GpSimd: GPSIMD instructions cannot access PSUM
                    nc.scalar.copy(osb, po[pp])
                    for bhalf in range(2):
                        b = 2 * pp + bhalf
                        nc.sync.dma_start(
                            out=out[b, hl, :, ic * 512 : (ic + 1) * 512],
                            in_=osb[64 * bhalf : 64 * (bhalf + 1), :],
                        )

            # software-pipelined emission: QK(gp) | EW(gp-1) | PV(gp-PV_LAG)
            PV_LAG = 3
            po_by_round = {}
            for gp in range(NGP + PV_LAG):
                if gp < NGP:
                    stage_qk(gp)
                if 0 <= gp - 1 < NGP:
                    stage_ew(gp - 1)
                pv = gp - PV_LAG
                if pv >= 0:
                    r = pv // NPAIR
                    if pv % NPAIR == 0:
                        po_by_round[r] = [
                            opsum.tile(
                                [128, 512], DT.float32, tag=f"po{q}", name=f"po{q}"
                            )
                            for q in range(2)
                        ]
                    stage_pv(pv, po_by_round[r])
                    if pv % NPAIR == NPAIR - 1:
                        flush_round(r, po_by_round.pop(r))

    # populate .instr bytes for InstISA subclasses (the custom DVE op);
    # without this the NEFF compiler sees empty .instr -> "ISA wrong length"
    from concourse.library_overlay import lower_extended_insts

    lower_extended_insts(nc)
    return nc


def _patch_bir_waits(bir_json: bytes) -> bytes:
    """This walrus build only accepts 1 sync wait per instruction (2 for
    DMACopy); Tile emits more. Legalize:
      1. merge duplicate-semaphore waits (keep max threshold),
      2. drop waits that are transitively implied (vector-clock replay over
         the straight-line program: in-order completion per engine, FIFO per
         DMA queue, and the knowledge a producer had when it bumped a sem),
      3. split any residual multi-wait onto injected EventSemaphore
         instructions on the same engine right before the instruction.
    Only monotonic sem-inc/sem-ge-imm semaphores participate in (2); barrier
    sems (dec/eq) are left untouched."""
    import json
    from collections import defaultdict

    bir = json.loads(bir_json)

    for fn in bir["functions"]:
        insts = []
        for bb in fn["blocks"]:
            for inst in bb.get("instructions", []):
                insts.append(inst)

        # classify sems: monotonic = all updates are positive sem-inc and
        # all waits are sem-ge-imm
        bad_sems = set()
        for inst in insts:
            si = inst.get("sync_info") or {}
            for u in si.get("on_update") or []:
                if u.get("update_mode") != "sem-inc" or u.get("update_value", 0) <= 0:
                    bad_sems.add(u["id"])
            for w in si.get("on_wait") or []:
                if w.get("wait_mode") != "sem-ge-imm":
                    bad_sems.add(w["id"])

        # proc of an instruction: its engine stream, except DMACopy whose
        # completion (and sem update) is FIFO per DMA queue, keyed by the
        # sem it updates.
        def proc_of(inst):
            if inst.get("opcode") == "DMACopy":
                si = inst.get("sync_info") or {}
                ups = si.get("on_update") or []
                if ups:
                    return ("dma", ups[0]["id"])
            return ("eng", inst.get("engine"))

        sem_val = defaultdict(int)          # current cumulative value per sem
        producers = defaultdict(list)       # sem -> [(value_after, CK dict)]
        know = defaultdict(dict)            # proc -> {sem: guaranteed min}

        def join(dst, src):
            for s, v in src.items():
                if dst.get(s, 0) < v:
                    dst[s] = v

        out_blocks = {id(bb): [] for bb in fn["blocks"]}
        inj = 0
        for bb in fn["blocks"]:
            new_list = []
            for inst in bb.get("instructions", []):
                p = proc_of(inst)
                eng_p = ("eng", inst.get("engine"))
                # waits on a DMACopy are enforced by the DGE queue (FIFO per
                # queue), not the issuing engine — track knowledge per queue
                kp = p if p[0] == "dma" else eng_p
                si = inst.get("sync_info") or {}
                waits = si.get("on_wait") or []
                # merge duplicate sems
                merged = {}
                for w in waits:
                    k = w["id"]
                    if k not in merged or w.get("wait_value", 0) > merged[k].get(
                        "wait_value", 0
                    ):
                        merged[k] = w
                waits = list(merged.values())
                kept = []
                for w in waits:
                    s, v = w["id"], w.get("wait_value", 0)
                    if s in bad_sems:
                        kept.append(w)
                        continue
                    if know[kp].get(s, 0) >= v:
                        continue  # redundant
                    kept.append(w)
                    know[kp][s] = max(know[kp].get(s, 0), v)
                    # transitive knowledge from the producer that reached v
                    for val_after, ck in producers[s]:
                        if val_after >= v:
                            join(know[kp], ck)
                            break
                # split if too many waits remain
                budget = 1
                while len(kept) > budget:
                    w = kept.pop(0)
                    inj += 1
                    new_list.append(
                        {
                            "debug": inst.get("debug", 0),
                            "engine": inst.get("engine"),
                            "ins": [],
                            "name": f"WS-{inj}-{inst.get('name')}",
                            "opcode": "EventSemaphore",
                            "outs": [],
                            "sync_info": {"on_update": [], "on_wait": [w]},
                        }
                    )
                si["on_wait"] = kept
                inst["sync_info"] = si
                new_list.append(inst)
                # apply this instruction's updates for downstream knowledge
                ups = si.get("on_update") or []
                ck = None
                for u in ups:
                    s = u["id"]
                    if s in bad_sems:
                        continue
                    sem_val[s] += u.get("update_value", 0)
                    if ck is None:
                        # completion knowledge: what this proc knew here
                        # (for DMA: queue knowledge + engine state at issue)
                        ck = dict(know[kp])
                        if p[0] == "dma":
                            join(ck, know[eng_p])
                    ck[s] = sem_val[s]
                    producers[s].append((sem_val[s], ck))
                # a proc knows its own sems' values after completion
                if p[0] == "eng":
                    for u in ups:
                        if u["id"] not in bad_sems:
                            know[eng_p][u["id"]] = sem_val[u["id"]]
            out_blocks[id(bb)] = new_list
        for bb in fn["blocks"]:
            bb["instructions"] = out_blocks[id(bb)]
    return json.dumps(bir).encode()


_PATCHED = False


def _install_bir_patch():
    global _PATCHED
    if _PATCHED:
        return
    import concourse.bass2jax as bass2jax
    from concourse import bass_utils as _bu

    orig = _bu.compile_bir_kernel

    def patched(bir_json, tmpdir, neff_name="file.neff"):
        return orig(_patch_bir_waits(bir_json), tmpdir, neff_name)

    bass2jax.compile_bir_kernel = patched
    # keep profile artifacts local — no bucket in this environment
    _bu.upload_artifacts = lambda tmpdir: str(tmpdir)
    _PATCHED = True


def _install_ntff_shim():
    """run_bass_kernel_spmd(trace=True) under axon needs
    antenv.axon_hooks.get_axon_ntff_profile_hook; the module isn't staged in
    this image, but libaxon_pjrt.so exposes the profile C ABI — recreate the
    shim (same recipe as trn_agent_boot)."""
    import sys as _sys

    if "antenv.axon_hooks" in _sys.modules:
        return
    import contextlib
    import ctypes
    import types

    import antenv  # noqa: F401

    so_path = "/opt/axon/libaxon_pjrt.so"
    hook = None
    try:
        lib = ctypes.CDLL(so_path)
        if hasattr(lib, "axon_start_nrt_profile"):
            lib.axon_start_nrt_profile.argtypes = [
                ctypes.POINTER(ctypes.c_int64),
                ctypes.c_size_t,
            ]
            lib.axon_start_nrt_profile.restype = ctypes.c_int64
            lib.axon_stop_nrt_profile.argtypes = [ctypes.c_char_p]
            lib.axon_stop_nrt_profile.restype = ctypes.c_int64

            @contextlib.contextmanager
            def hook(output_dir, device_ids):
                import jax

                jax.devices()
                if device_ids:
                    ids = (ctypes.c_int64 * len(device_ids))(*device_ids)
                    rc = lib.axon_start_nrt_profile(ids, len(device_ids))
                else:
                    rc = lib.axon_start_nrt_profile(None, 0)
                if rc != 0:
                    raise RuntimeError(f"axon_start_nrt_profile rc={rc}")
                try:
                    yield
                finally:
                    n = lib.axon_stop_nrt_profile(str(output_dir).encode())
                    print(
                        f"ntff profile: {n} file(s) -> {output_dir}",
                        file=_sys.stderr,
                    )
    except OSError:
        pass

    mod = types.ModuleType("antenv.axon_hooks")
    mod.get_axon_ntff_profile_hook = lambda: hook
    mod.set_axon_ntff_profile_hook = lambda h: None
    _sys.modules["antenv.axon_hooks"] = mod
    import antenv as _ae

    _ae.axon_hooks = mod


def kernel(query, key, value, mask=None):
    global _NC, LAST_EXEC_NS, LAST_RESULTS
    from concourse.bass_utils import run_bass_kernel_spmd

    _install_bir_patch()
    if TRACE:
        _install_ntff_shim()

    query = np.asarray(query, dtype=np.float32)
    key = np.asarray(key, dtype=np.float32)
    value = np.asarray(value, dtype=np.float32)

    if _NC is None:
        _NC = _build_nc()
    nc = _NC

    bf16 = ml_dtypes.bfloat16

    def pack_pivot(x, negate_base):
        # [B, HL, S, D] -> [B, HL, D, S]; stack [x_b^T ; (+-)x_0^T] on the
        # partition axis for b = 1..3 -> [3, HL, 128, S]
        xt = x.transpose(0, 1, 3, 2)  # [B, HL, D, S]
        base = -xt[0] if negate_base else xt[0]  # [HL, D, S]
        stk = np.stack(
            [np.concatenate([xt[b], base], axis=1) for b in (1, 2, 3)], axis=0
        )
        return np.ascontiguousarray(stk).astype(bf16)

    in_maps = []
    for c in range(NCORES):
        hs = slice(HL * c, HL * (c + 1))
        qtc = pack_pivot(query[:, hs], negate_base=True)
        ktc = pack_pivot(key[:, hs], negate_base=False)
        vc = np.ascontiguousarray(value[:, hs]).astype(bf16)
        in_maps.append({"qt": qtc, "kt": ktc, "v": vc})

    res = run_bass_kernel_spmd(
        nc, in_maps, core_ids=list(range(NCORES)), trace=TRACE
    )
    LAST_RESULTS = res
    LAST_EXEC_NS = getattr(res, "exec_time_ns", None)

    full = np.empty((B, H, S, D), dtype=np.float32)
    for c in range(NCORES):
        o = np.asarray(res.results[c]["out"])  # [B, HL, D, S]
        full[:, HL * c : HL * (c + 1)] = o.transpose(0, 1, 3, 2)
    return full


# revision 21
# speedup vs baseline: 1.2031x; 1.0139x over previous
"""Trainium2 Bass kernel for nn_Attention_10711648436709.

Math (faithful to reference):
    h = einsum('bhik,bhjk->bhij', Q, K) / sqrt(H)     # scale = sqrt(16) = 4
    w = softmax(h, axis=0)                            # over the BATCH axis (B=4)
    out = einsum('bhij,bhjv->bhiv', w, V)
    (mask is a no-op in the reference)

Sharding: head-parallel across 8 cores (16 heads -> 2 heads/core).
Softmax over batch stays core-local => communication-free.

Per-core layout: compute transposed scores S^T[j, i]:
 - QK:  lhsT = [K_b^T ; K_0^T][d, j-block]  rhs = [Q_b^T ; -Q_0^T][d, i-chunk]
   (host packs/negates) -> one K=128 matmul per b in {1,2,3} yields
   g_b = h_b - h_0 directly (batch-0-pivot softmax).
 - w_b = e^{g_b/4} * r, w_0 = r, r = 1/(1 + sum_b e^{g_b/4}).
 - PV:  lhsT = V[j-block, v]  rhs = W_b[j, i-chunk]; out^T[v,i] accumulates
   in PSUM over the 16 j-blocks; host transposes back.

Engine split (v2): exp on ACT; E1+E2 and the W=E*r multiply on DVE (2x bf16);
den = (E3+1)+T1 on GpSimd; reciprocal via the fused RECIPROCAL_APPROX_FAST
custom DVE op (1 instr) with a fraction routed to ACT (Ln+Exp) for balance.
Elementwise ops batched over PAIRS of j-blocks ([128,1024] tiles) to amortize
per-instruction overheads. Emission is software-pipelined: QK runs 2 pairs
ahead of PV so the tensor engine stays continuously fed (p-state ramp).
"""

import sys
import os

for p in ("/opt/trn_rl_repo",):
    if p not in sys.path:
        sys.path.insert(0, p)

import numpy as np
import ml_dtypes

B, H, S, D = 4, 16, 2048, 64
NCORES = 8
HL = H // NCORES          # 2 heads per core
NB = S // 128             # 16 j-blocks
NI = S // 512             # 4 i-chunks
NPAIR = NB // 2           # 8 j-block pairs per (hl, ic) round
NROUND = HL * NI          # 8 rounds
NGP = NROUND * NPAIR      # 64 global pairs

# every ACT_EVERY-th pair computes r on ACT (Ln+Exp) instead of DVE
ACT_EVERY = 6

TRACE = False
LAST_EXEC_NS = None
LAST_RESULTS = None

_NC = None


def _build_nc():
    import concourse.bass as bass
    import concourse.mybir as mybir
    import concourse.tile as tile
    from concourse.dve_ops import (
        RECIP_APPROX_FAST_CONSTS,
        RECIPROCAL_APPROX_FAST,
    )

    DT = mybir.dt
    AF = mybir.ActivationFunctionType
    ALU = mybir.AluOpType
    RC = RECIP_APPROX_FAST_CONSTS

    nc = bass.Bass()
    qt = nc.declare_dram_parameter("qt", [3, HL, 128, S], DT.bfloat16, isOutput=False)
    kt = nc.declare_dram_parameter("kt", [3, HL, 128, S], DT.bfloat16, isOutput=False)
    vv = nc.declare_dram_parameter("v", [B, HL, S, D], DT.bfloat16, isOutput=False)
    out = nc.declare_dram_parameter("out", [B, HL, D, S], DT.float32, isOutput=True)

    with tile.TileContext(nc) as tc:
        with (
            tc.tile_pool(name="inputs", bufs=1) as ipool,
            tc.tile_pool(name="work", bufs=7) as wpool,
            # rb lives until PV consumes it PV_LAG pairs later -> needs
            # bufs > PV_LAG + chain depth or the recip write WAR-stalls on PV
            tc.tile_pool(name="rbp", bufs=7) as rbpool,
            tc.tile_pool(name="mid", bufs=3) as mpool,
            tc.tile_pool(name="lnp", bufs=2) as lnpool,
            tc.tile_pool(name="outsb", bufs=4) as opool,
            tc.tile_pool(name="qkps", bufs=2, space="PSUM") as qkpool,
            tc.tile_pool(name="ops", bufs=1, space="PSUM") as opsum,
        ):
            QT = ipool.tile([128, 3 * HL * S], DT.bfloat16, tag="qt")
            KT = ipool.tile([128, 3 * HL * S], DT.bfloat16, tag="kt")
            VA = ipool.tile([128, B * HL * NB * D], DT.bfloat16, tag="va")
            # hl-major so the first (hl=0) round's operands land first
            for hl in range(HL):
                for bb in range(3):
                    off = (bb * HL + hl) * S
                    nc.sync.dma_start(out=KT[:, off : off + S], in_=kt[bb, hl])
                    nc.sync.dma_start(out=QT[:, off : off + S], in_=qt[bb, hl])
                for b in range(B):
                    voff = (b * HL + hl) * NB * D
                    nc.sync.dma_start(
                        out=VA[:, voff : voff + NB * D].rearrange(
                            "p (n d) -> p n d", d=D
                        ),
                        in_=vv[b, hl].rearrange("(n p) d -> p n d", p=128),
                    )

            # per-global-pair state for the software pipeline
            st = {}

            def rnd(gp):
                r = gp // NPAIR
                return r // NI, r % NI  # (hl, ic)

            def stage_qk(gp):
                """QK matmuls + exp for both j-blocks of pair gp."""
                hl, ic = rnd(gp)
                p = gp % NPAIR
                # E pair tile, plane-major: [b=3, pair-interleaved 1024]
                E = wpool.tile([128, 3072], DT.bfloat16, tag="E")
                for half in range(2):
                    jb = 2 * p + half
                    qk = qkpool.tile([128, 1536], DT.float32, tag="qk")
                    for bb in range(3):
                        off = (bb * HL + hl) * S
                        nc.tensor.matmul(
                            qk[:, bb * 512 : (bb + 1) * 512],
                            lhsT=KT[:, off + jb * 128 : off + jb * 128 + 128],
                            rhs=QT[:, off + ic * 512 : off + ic * 512 + 512],
                            start=True,
                            stop=True,
                        )
                    # exp of all 3 planes into interleaved pair layout:
                    # plane b lives at columns [b*1024 + half*512, +512)
                    E3 = E.rearrange("q (b n) -> q b n", b=3)
                    nc.scalar.activation(
                        E3[:, :, half * 512 : half * 512 + 512],
                        qk.rearrange("q (b n) -> q b n", b=3),
                        AF.Exp,
                        scale=0.25,
                    )
                st[gp] = {"E": E}

            def stage_ew(gp):
                """T1/den/recip/W for pair gp (runs one pair behind QK)."""
                E = st[gp]["E"]
                T1 = mpool.tile([128, 1024], DT.bfloat16, tag="T1")
                nc.vector.tensor_add(T1, E[:, 0:1024], E[:, 1024:2048])
                # fold the softmax +1 in before the (slow) gpsimd hop so the
                # gpsimd add feeds the reciprocal directly
                T2 = mpool.tile([128, 1024], DT.bfloat16, tag="T2")
                nc.vector.tensor_scalar(
                    out=T2,
                    in0=T1,
                    scalar1=1.0,
                    scalar2=None,
                    op0=ALU.add,
                )
                # GpSimd supports plain TensorTensor only (no STT, no PSUM)
                den = mpool.tile([128, 1024], DT.bfloat16, tag="den")
                nc.gpsimd.tensor_add(den, E[:, 2048:3072], T2)
                rb = rbpool.tile([128, 1024], DT.bfloat16, tag="rb")
                if gp % ACT_EVERY == ACT_EVERY - 1:
                    # reciprocal on ACT: r = exp(-ln(den)); Ln and Exp share
                    # the natural_log_exp table set
                    lnt = lnpool.tile([128, 1024], DT.bfloat16, tag="lnt")
                    nc.scalar.activation(lnt, den, AF.Ln)
                    nc.scalar.activation(rb, lnt, AF.Exp, scale=-1.0)
                else:
                    # fused seed + 2 Newton passes, one DVE instruction
                    nc.vector._custom_dve(
                        RECIPROCAL_APPROX_FAST,
                        out=rb,
                        in0=den,
                        s0=RC["s0"],
                        s1=RC["s1"],
                        imm2=RC["imm2"],
                    )
                W = wpool.tile([128, 3072], DT.bfloat16, tag="W")
                nc.vector.tensor_mul(
                    W.rearrange("q (b n) -> q b n", b=3),
                    E.rearrange("q (b n) -> q b n", b=3),
                    rb.unsqueeze(1).broadcast_to([128, 3, 1024]),
                )
                st[gp].update({"rb": rb, "W": W})

            def stage_pv(gp, po):
                """PV accumulation for both j-blocks of pair gp."""
                hl, ic = rnd(gp)
                p = gp % NPAIR
                rb, W = st[gp]["rb"], st[gp]["W"]
                for half in range(2):
                    jb = 2 * p + half
                    rhss = [
                        rb[:, half * 512 : half * 512 + 512],
                        W[:, half * 512 : half * 512 + 512],
                        W[:, 1024 + half * 512 : 1024 + half * 512 + 512],
                        W[:, 2048 + half * 512 : 2048 + half * 512 + 512],
                    ]
                    for pp in range(2):
                        for bhalf in range(2):
                            b = 2 * pp + bhalf
                            voff = (b * HL + hl) * NB * D + jb * D
                            nc.tensor.matmul(
                                po[pp][64 * bhalf : 64 * (bhalf + 1), :],
                                lhsT=VA[:, voff : voff + D],
                                rhs=rhss[b],
                                start=(jb == 0),
                                stop=(jb == NB - 1),
                                tile_position=(0, 64 * bhalf),
                            )
                del st[gp]

            def flush_round(r, po):
                """PSUM -> SBUF -> HBM for round r's outputs."""
                hl, ic = r // NI, r % NI
                for pp in range(2):
                    osb = opool.tile([128, 512], DT.float32, tag=f"osb{pp}")
                    # ACT, not GpSimd: GPSIMD instructions cannot access PSUM
                    nc.scalar.copy(osb, po[pp])
                    for bhalf in range(2):
                        b = 2 * pp + bhalf
                        nc.sync.dma_start(
                            out=out[b, hl, :, ic * 512 : (ic + 1) * 512],
                            in_=osb[64 * bhalf : 64 * (bhalf + 1), :],
                        )

            # software-pipelined emission: QK(gp) | EW(gp-1) | PV(gp-PV_LAG)
            PV_LAG = 4
            po_by_round = {}
            for gp in range(NGP + PV_LAG):
                if gp < NGP:
                    stage_qk(gp)
                if 0 <= gp - 1 < NGP:
                    stage_ew(gp - 1)
                pv = gp - PV_LAG
                if pv >= 0:
                    r = pv // NPAIR
                    if pv % NPAIR == 0:
                        po_by_round[r] = [
                            opsum.tile(
                                [128, 512], DT.float32, tag=f"po{q}", name=f"po{q}"
                            )
                            for q in range(2)
                        ]
                    stage_pv(pv, po_by_round[r])
                    if pv % NPAIR == NPAIR - 1:
                        flush_round(r, po_by_round.pop(r))

    # populate .instr bytes for InstISA subclasses (the custom DVE op);
    # without this the NEFF compiler sees empty .instr -> "ISA wrong length"
    from concourse.library_overlay import lower_extended_insts

    lower_extended_insts(nc)
    return nc


def _patch_bir_waits(bir_json: bytes) -> bytes:
    """This walrus build only accepts 1 sync wait per instruction (2 for
    DMACopy); Tile emits more. Legalize:
      1. merge duplicate-semaphore waits (keep max threshold),
      2. drop waits that are transitively implied (vector-clock replay over
         the straight-line program: in-order completion per engine, FIFO per
         DMA queue, and the knowledge a producer had when it bumped a sem),
      3. split any residual multi-wait onto injected EventSemaphore
         instructions on the same engine right before the instruction.
    Only monotonic sem-inc/sem-ge-imm semaphores participate in (2); barrier
    sems (dec/eq) are left untouched."""
    import json
    from collections import defaultdict

    bir = json.loads(bir_json)

    for fn in bir["functions"]:
        insts = []
        for bb in fn["blocks"]:
            for inst in bb.get("instructions", []):
                insts.append(inst)

        # classify sems: monotonic = all updates are positive sem-inc and
        # all waits are sem-ge-imm
        bad_sems = set()
        for inst in insts:
            si = inst.get("sync_info") or {}
            for u in si.get("on_update") or []:
                if u.get("update_mode") != "sem-inc" or u.get("update_value", 0) <= 0:
                    bad_sems.add(u["id"])
            for w in si.get("on_wait") or []:
                if w.get("wait_mode") != "sem-ge-imm":
                    bad_sems.add(w["id"])

        # proc of an instruction: its engine stream, except DMACopy whose
        # completion (and sem update) is FIFO per DMA queue, keyed by the
        # sem it updates.
        def proc_of(inst):
            if inst.get("opcode") == "DMACopy":
                si = inst.get("sync_info") or {}
                ups = si.get("on_update") or []
                if ups:
                    return ("dma", ups[0]["id"])
            return ("eng", inst.get("engine"))

        sem_val = defaultdict(int)          # current cumulative value per sem
        producers = defaultdict(list)       # sem -> [(value_after, CK dict)]
        know = defaultdict(dict)            # proc -> {sem: guaranteed min}

        def join(dst, src):
            for s, v in src.items():
                if dst.get(s, 0) < v:
                    dst[s] = v

        out_blocks = {id(bb): [] for bb in fn["blocks"]}
        inj = 0
        for bb in fn["blocks"]:
            new_list = []
            for inst in bb.get("instructions", []):
                p = proc_of(inst)
                eng_p = ("eng", inst.get("engine"))
                # waits on a DMACopy are enforced by the DGE queue (FIFO per
                # queue), not the issuing engine — track knowledge per queue
                kp = p if p[0] == "dma" else eng_p
                si = inst.get("sync_info") or {}
                waits = si.get("on_wait") or []
                # merge duplicate sems
                merged = {}
                for w in waits:
                    k = w["id"]
                    if k not in merged or w.get("wait_value", 0) > merged[k].get(
                        "wait_value", 0
                    ):
                        merged[k] = w
                waits = list(merged.values())
                kept = []
                for w in waits:
                    s, v = w["id"], w.get("wait_value", 0)
                    if s in bad_sems:
                        kept.append(w)
                        continue
                    if know[kp].get(s, 0) >= v:
                        continue  # redundant
                    kept.append(w)
                    know[kp][s] = max(know[kp].get(s, 0), v)
                    # transitive knowledge from the producer that reached v
                    for val_after, ck in producers[s]:
                        if val_after >= v:
                            join(know[kp], ck)
                            break
                # split if too many waits remain
                budget = 1
                while len(kept) > budget:
                    w = kept.pop(0)
                    inj += 1
                    new_list.append(
                        {
                            "debug": inst.get("debug", 0),
                            "engine": inst.get("engine"),
                            "ins": [],
                            "name": f"WS-{inj}-{inst.get('name')}",
                            "opcode": "EventSemaphore",
                            "outs": [],
                            "sync_info": {"on_update": [], "on_wait": [w]},
                        }
                    )
                si["on_wait"] = kept
                inst["sync_info"] = si
                new_list.append(inst)
                # apply this instruction's updates for downstream knowledge
                ups = si.get("on_update") or []
                ck = None
                for u in ups:
                    s = u["id"]
                    if s in bad_sems:
                        continue
                    sem_val[s] += u.get("update_value", 0)
                    if ck is None:
                        # completion knowledge: what this proc knew here
                        # (for DMA: queue knowledge + engine state at issue)
                        ck = dict(know[kp])
                        if p[0] == "dma":
                            join(ck, know[eng_p])
                    ck[s] = sem_val[s]
                    producers[s].append((sem_val[s], ck))
                # a proc knows its own sems' values after completion
                if p[0] == "eng":
                    for u in ups:
                        if u["id"] not in bad_sems:
                            know[eng_p][u["id"]] = sem_val[u["id"]]
            out_blocks[id(bb)] = new_list
        for bb in fn["blocks"]:
            bb["instructions"] = out_blocks[id(bb)]
    return json.dumps(bir).encode()


_PATCHED = False


def _install_bir_patch():
    global _PATCHED
    if _PATCHED:
        return
    import concourse.bass2jax as bass2jax
    from concourse import bass_utils as _bu

    orig = _bu.compile_bir_kernel

    def patched(bir_json, tmpdir, neff_name="file.neff"):
        return orig(_patch_bir_waits(bir_json), tmpdir, neff_name)

    bass2jax.compile_bir_kernel = patched
    # keep profile artifacts local — no bucket in this environment
    _bu.upload_artifacts = lambda tmpdir: str(tmpdir)
    _PATCHED = True


def _install_ntff_shim():
    """run_bass_kernel_spmd(trace=True) under axon needs
    antenv.axon_hooks.get_axon_ntff_profile_hook; the module isn't staged in
    this image, but libaxon_pjrt.so exposes the profile C ABI — recreate the
    shim (same recipe as trn_agent_boot)."""
    import sys as _sys

    if "antenv.axon_hooks" in _sys.modules:
        return
    import contextlib
    import ctypes
    import types

    import antenv  # noqa: F401

    so_path = "/opt/axon/libaxon_pjrt.so"
    hook = None
    try:
        lib = ctypes.CDLL(so_path)
        if hasattr(lib, "axon_start_nrt_profile"):
            lib.axon_start_nrt_profile.argtypes = [
                ctypes.POINTER(ctypes.c_int64),
                ctypes.c_size_t,
            ]
            lib.axon_start_nrt_profile.restype = ctypes.c_int64
            lib.axon_stop_nrt_profile.argtypes = [ctypes.c_char_p]
            lib.axon_stop_nrt_profile.restype = ctypes.c_int64

            @contextlib.contextmanager
            def hook(output_dir, device_ids):
                import jax

                jax.devices()
                if device_ids:
                    ids = (ctypes.c_int64 * len(device_ids))(*device_ids)
                    rc = lib.axon_start_nrt_profile(ids, len(device_ids))
                else:
                    rc = lib.axon_start_nrt_profile(None, 0)
                if rc != 0:
                    raise RuntimeError(f"axon_start_nrt_profile rc={rc}")
                try:
                    yield
                finally:
                    n = lib.axon_stop_nrt_profile(str(output_dir).encode())
                    print(
                        f"ntff profile: {n} file(s) -> {output_dir}",
                        file=_sys.stderr,
                    )
    except OSError:
        pass

    mod = types.ModuleType("antenv.axon_hooks")
    mod.get_axon_ntff_profile_hook = lambda: hook
    mod.set_axon_ntff_profile_hook = lambda h: None
    _sys.modules["antenv.axon_hooks"] = mod
    import antenv as _ae

    _ae.axon_hooks = mod


def kernel(query, key, value, mask=None):
    global _NC, LAST_EXEC_NS, LAST_RESULTS
    from concourse.bass_utils import run_bass_kernel_spmd

    _install_bir_patch()
    if TRACE:
        _install_ntff_shim()

    query = np.asarray(query, dtype=np.float32)
    key = np.asarray(key, dtype=np.float32)
    value = np.asarray(value, dtype=np.float32)

    if _NC is None:
        _NC = _build_nc()
    nc = _NC

    bf16 = ml_dtypes.bfloat16

    def pack_pivot(x, negate_base):
        # [B, HL, S, D] -> [B, HL, D, S]; stack [x_b^T ; (+-)x_0^T] on the
        # partition axis for b = 1..3 -> [3, HL, 128, S]
        xt = x.transpose(0, 1, 3, 2)  # [B, HL, D, S]
        base = -xt[0] if negate_base else xt[0]  # [HL, D, S]
        stk = np.stack(
            [np.concatenate([xt[b], base], axis=1) for b in (1, 2, 3)], axis=0
        )
        return np.ascontiguousarray(stk).astype(bf16)

    in_maps = []
    for c in range(NCORES):
        hs = slice(HL * c, HL * (c + 1))
        qtc = pack_pivot(query[:, hs], negate_base=True)
        ktc = pack_pivot(key[:, hs], negate_base=False)
        vc = np.ascontiguousarray(value[:, hs]).astype(bf16)
        in_maps.append({"qt": qtc, "kt": ktc, "v": vc})

    res = run_bass_kernel_spmd(
        nc, in_maps, core_ids=list(range(NCORES)), trace=TRACE
    )
    LAST_RESULTS = res
    LAST_EXEC_NS = getattr(res, "exec_time_ns", None)

    full = np.empty((B, H, S, D), dtype=np.float32)
    for c in range(NCORES):
        o = np.asarray(res.results[c]["out"])  # [B, HL, D, S]
        full[:, HL * c : HL * (c + 1)] = o.transpose(0, 1, 3, 2)
    return full
